# revision 49
# baseline (speedup 1.0000x reference)
"""Trainium2 Bass kernel for nn_DecodeMoeOps (MoE decode: dispatch-quant,
grouped int8 GEMM1, SwiGLU, requant, grouped int8 GEMM2, weighted combine).

Expert-parallel across 8 NeuronCores: core c owns experts {2c, 2c+1}. Each
core computes only the tokens routed to its experts (gathered host-side,
padded to N_PAD), using weight-stationary GEMMs over routed tokens:

  GEMM1: out[f, tok] = w1_tile[k,f].T @ xqs[k, tok]   (xqs = fp16(xq*sx))
  GEMM2: out[h, tok] = w2_tile[i,h].T @ aq[i, tok]

w1 ships as int8 and is cast to fp16 on-chip (split across DVE/ACT/GPSIMD);
w2 ships as fp8e3m4 (exact for |w|<=31, max abs err 2 above) with the 16x
scale folded into w2_scale. The last expert's w2 streams as a 12-chunk-wide
stream then a 4-chunk-wide stream, each with its own PSUM group, so the
dependent chain after the final weight bytes is only 4 matmuls + a small
dequant + a 512B output DMA. Output DMAs are queued on the SP ring after all
weight DMAs so they ride the tail's dead window. Host scatters the
per-expert [h, tok] outputs back into y[B, H].
"""

import os
import sys

for _p in ("/opt/trn_rl_repo", "/root/.axon_site/_ro/trn_rl_repo"):
    if os.path.isdir(_p) and _p not in sys.path:
        sys.path.insert(0, _p)

from contextlib import ExitStack

import ml_dtypes
import numpy as np

import concourse.bass as bass
import concourse.mybir as mybir
import concourse.tile as tile
from concourse import bacc
from concourse import bass_isa
from concourse.bass_utils import run_bass_kernel_spmd

B, TOPK, H, I, E = 128, 8, 2048, 1408, 16
NCORES = 8
EPC = E // NCORES  # experts per core
KH = H // 128  # 16 k-tiles for GEMM1 contraction
KI = I // 128  # 11 k-tiles for GEMM2 contraction
FT = I // 128  # 11 f-tiles per GEMM1 half
HT = H // 128  # 16 h-tiles for GEMM2 output
I2 = 2 * I
F32 = mybir.dt.float32
BF16 = mybir.dt.bfloat16
F16 = mybir.dt.float16
I8 = mybir.dt.int8
F8E3 = mybir.dt.float8e3
MAGIC = float(3 * 2**22)  # fp32 round-to-int magic (covers negatives)

# on-chip int8->fp16 cast: free-dim split of each [128, 2816] w1 k-tile,
# proportional to effective engine rates (DVE ~1.79, ACT ~0.91, Pool ~0.64
# cols/ns) so the three casts finish together within the 1001ns tile window
CAST_DVE = (0, 1472)
CAST_ACT = (1472, 800)
CAST_POOL = (2272, 544)

# last expert's GEMM2 h-split: wide stream then a short tail stream
HA = 12  # h-chunks in the wide stream
HB = HT - HA  # h-chunks in the tail stream

# packed small-tensor layout (per expert, per partition): columns
MC_S1G = 0
MC_S1GU = FT
MC_W2S = 2 * FT
MC_COMB = 2 * FT + HT  # comb occupies n_pad cols

_cache: dict = {}


def _build_program(n_pad: int):
    mult = mybir.AluOpType.mult
    nc = bacc.Bacc(
        "TRN2",
        target_bir_lowering=False,
        debug=False,
        num_devices=NCORES,
    )

    FW = FT * n_pad   # gate/up accumulator width
    HW = HT * n_pad   # GEMM2 output width (bf16 cols per expert)
    MC = 2 * FT + HT + n_pad  # packed meta cols per expert

    # --- per-core DRAM I/O ---
    xqsT_d = nc.dram_tensor("xqsT", [EPC, 128, KH, n_pad], F16, kind="ExternalInput").ap()
    w1_d = nc.dram_tensor("w1t", [EPC, KH, 128, I2], I8, kind="ExternalInput").ap()
    w2_d = nc.dram_tensor("w2t", [EPC, KI, 128, H], F8E3, kind="ExternalInput").ap()
    meta_d = nc.dram_tensor("meta", [EPC, 128, MC], F32, kind="ExternalInput").ap()
    o_d = nc.dram_tensor("o", [EPC, 128, HW], BF16, kind="ExternalOutput").ap()

    with tile.TileContext(nc) as tc, ExitStack() as ctx:
        consts = ctx.enter_context(tc.tile_pool(name="consts", bufs=1))
        w1i8p = ctx.enter_context(tc.tile_pool(name="w1i8", bufs=7))
        w1f16p = ctx.enter_context(tc.tile_pool(name="w1f16", bufs=6))
        w2fp = ctx.enter_context(tc.tile_pool(name="w2f", bufs=1))
        w2ap = ctx.enter_context(tc.tile_pool(name="w2a", bufs=1))
        w2bp = ctx.enter_context(tc.tile_pool(name="w2b", bufs=1))
        epi = ctx.enter_context(tc.tile_pool(name="epi", bufs=2))
        op_ = ctx.enter_context(tc.tile_pool(name="op", bufs=1))
        ps1_pool = ctx.enter_context(tc.tile_pool(name="ps1", bufs=1, space="PSUM"))
        ps2f_pool = ctx.enter_context(tc.tile_pool(name="ps2f", bufs=1, space="PSUM"))
        ps2a_pool = ctx.enter_context(tc.tile_pool(name="ps2a", bufs=1, space="PSUM"))
        ps2b_pool = ctx.enter_context(tc.tile_pool(name="ps2b", bufs=1, space="PSUM"))

        # --- prologue: small inputs on the ACT queue ---
        xqs_s = consts.tile([128, EPC, KH, n_pad], F16, name="xqs_s")
        nc.scalar.dma_start(out=xqs_s[:], in_=xqsT_d.rearrange("e p k j -> p e k j"))
        meta_s = consts.tile([128, EPC, MC], F32, name="meta_s")
        nc.scalar.dma_start(out=meta_s[:], in_=meta_d.rearrange("e p c -> p e c"))
        s1g_s = meta_s[:, :, MC_S1G : MC_S1G + FT]
        s1gu_s = meta_s[:, :, MC_S1GU : MC_S1GU + FT]
        w2s_s = meta_s[:, :, MC_W2S : MC_W2S + HT]
        comb_s = meta_s[:, :, MC_COMB : MC_COMB + n_pad]

        # per-channel GEMM1 scales broadcast along tokens, materialized at
        # program start on ACT (whose first-expert cast window has slack --
        # putting these on DVE would stall its whole cast queue behind the
        # meta DMA)
        s1g_b = consts.tile([128, EPC, FT, n_pad], F32, name="s1g_b")
        s1gu_b = consts.tile([128, EPC, FT, n_pad], F32, name="s1gu_b")
        for e in range(EPC):
            if e == 0:
                nc.gpsimd.tensor_copy(
                    out=s1g_b[:, e],
                    in_=s1g_s[:, e, :].unsqueeze(2).broadcast_to([128, FT, n_pad]))
                nc.gpsimd.tensor_copy(
                    out=s1gu_b[:, e],
                    in_=s1gu_s[:, e, :].unsqueeze(2).broadcast_to([128, FT, n_pad]))
            else:
                nc.scalar.activation(
                    out=s1g_b[:, e],
                    in_=s1g_s[:, e, :].unsqueeze(2).broadcast_to([128, FT, n_pad]),
                    func=mybir.ActivationFunctionType.Copy)
                nc.scalar.activation(
                    out=s1gu_b[:, e],
                    in_=s1gu_s[:, e, :].unsqueeze(2).broadcast_to([128, FT, n_pad]),
                    func=mybir.ActivationFunctionType.Copy)

        def bank_flags(offsets_bytes):
            """PSUM accumulation start/stop flags per chunk: matmul start=True
            zeroes the whole 2KB bank, so exactly one start (first chunk) and
            one stop (last chunk) per bank. Offsets must not cross banks."""
            first, last = {}, {}
            for i, off in enumerate(offsets_bytes):
                b = off // 2048
                if b not in first:
                    first[b] = i
                last[b] = i
            starts = {i for i in first.values()}
            stops = {i for i in last.values()}
            return starts, stops

        def epilogue1_ops(e, ps1_e, out, pipelined=False):
            """dequant + SwiGLU + requant -> aq; returns one closure per op
            so the caller can interleave emission with other work. gate (DVE)
            and up2 (GPSIMD) both read ps1 immediately so the single ps1
            buffer frees ~0.8us after GEMM1 stops.

            pipelined=True (last expert) halves the SwiGLU stages and chunks
            the requant per ki-group so the first aq chunks land ~4us after
            GEMM1 stops instead of ~7us -- GEMM2 must never leave the PE
            idle long enough to reset its clock ramp."""
            ps_g = ps1_e[:, 0:FW]
            ps_u = ps1_e[:, FW : 2 * FW]
            gate = epi.tile([128, FW], F32, tag="gate", name=f"gate_{e}")
            up2 = epi.tile([128, FW], F32, tag="up2", name=f"up2_{e}")
            gs = epi.tile([128, FW], F32, tag="gs", name=f"gs_{e}")
            act2 = epi.tile([128, FW], F32, tag="act2", name=f"act2_{e}")
            am = epi.tile([128, FW], F32, tag="am", name=f"am_{e}")
            m = epi.tile([128, n_pad], F32, tag="m", name=f"m_{e}")
            mc = epi.tile([128, n_pad], F32, tag="mc", name=f"mc_{e}")
            r = epi.tile([128, n_pad], F32, tag="r", name=f"r_{e}")
            tq = epi.tile([128, FW], F32, tag="tq", name=f"tq_{e}")
            tq2 = epi.tile([128, FW], F32, tag="tq2", name=f"tq2_{e}")
            aq = epi.tile([128, FT, n_pad], BF16, tag="aq", name=f"aq_{e}")
            s2c = epi.tile([128, n_pad], F32, tag="s2c", name=f"s2c_{e}")
            w2sc = epi.tile([128, HT, n_pad], F32, tag="w2sc", name=f"w2sc_{e}")
            out["aq"], out["w2sc"], out["s2c"] = aq, w2sc, s2c
            ops = []

            # SwiGLU stages; quartered for the pipelined (last) expert so the
            # DVE/ACT/GPSIMD chain overlaps itself. up2 rides Pool for the
            # interleaved expert but DVE for the pipelined one -- Pool's cast
            # backlog would otherwise gate the whole aq chain.
            parts = [(0, FT)] if not pipelined else [(0, 6), (6, FT)]

            def rng(buf, t0, t1):
                return buf[:].rearrange("p (t n) -> p t n", t=FT)[:, t0:t1, :]

            # GPSIMD cannot read PSUM -- up2 must ride DVE (or ACT)
            up_eng = nc.vector
            for pi, (t0, t1) in enumerate(parts):
                ops += [
                    lambda t0=t0, t1=t1: nc.vector.tensor_tensor(
                        out=rng(gate, t0, t1),
                        in0=ps_g.rearrange("p (t n) -> p t n", t=FT)[:, t0:t1, :],
                        in1=s1g_b[:, e, t0:t1, :], op=mult),
                    lambda t0=t0, t1=t1: up_eng.tensor_tensor(
                        out=rng(up2, t0, t1),
                        in0=ps_u.rearrange("p (t n) -> p t n", t=FT)[:, t0:t1, :],
                        in1=s1gu_b[:, e, t0:t1, :], op=mult),
                    # silu(gate) = gate * sigmoid(gate) in one ACT op
                    lambda t0=t0, t1=t1: nc.scalar.activation(
                        out=gs[:, t0 * n_pad : t1 * n_pad],
                        in_=gate[:, t0 * n_pad : t1 * n_pad],
                        func=mybir.ActivationFunctionType.Silu),
                    # act2 inputs live in SBUF -> Pool, keeping DVE's
                    # in-order queue short (dequants must not queue behind)
                    lambda t0=t0, t1=t1: nc.gpsimd.tensor_tensor(
                        out=act2[:, t0 * n_pad : t1 * n_pad],
                        in0=gs[:, t0 * n_pad : t1 * n_pad],
                        in1=up2[:, t0 * n_pad : t1 * n_pad], op=mult),
                    lambda t0=t0, t1=t1: nc.gpsimd.partition_all_reduce(
                        am[:, t0 * n_pad : t1 * n_pad],
                        act2[:, t0 * n_pad : t1 * n_pad], channels=128,
                        reduce_op=bass_isa.ReduceOp.absmax),
                ]
            ops += [
                lambda: nc.vector.tensor_reduce(
                    out=m[:], in_=am[:].rearrange("p (t n) -> p n t", t=FT),
                    op=mybir.AluOpType.max, axis=mybir.AxisListType.X),
                lambda: nc.vector.tensor_scalar_max(
                    out=mc[:], in0=m[:], scalar1=1e-12),
                lambda: nc.vector.reciprocal(out=r[:], in_=mc[:]),
            ]
            kgroups = [(0, FT)] if not pipelined else [(0, 6), (6, FT)]
            for k0, k1 in kgroups:
                ops += [
                    lambda k0=k0, k1=k1: nc.vector.scalar_tensor_tensor(
                        out=rng(tq, k0, k1),
                        in0=rng(act2, k0, k1),
                        scalar=127.0,
                        in1=r[:].unsqueeze(1).broadcast_to([128, k1 - k0, n_pad]),
                        op0=mult, op1=mult),
                    lambda k0=k0, k1=k1: nc.scalar.activation(
                        out=tq2[:, k0 * n_pad : k1 * n_pad],
                        in_=tq[:, k0 * n_pad : k1 * n_pad],
                        func=mybir.ActivationFunctionType.Copy, bias=MAGIC),
                    lambda k0=k0, k1=k1: nc.vector.tensor_scalar_add(
                        out=aq[:, k0:k1, :].rearrange("p t n -> p (t n)"),
                        in0=tq2[:, k0 * n_pad : k1 * n_pad],
                        scalar1=-MAGIC),
                ]
            ops += [
                lambda: nc.vector.scalar_tensor_tensor(
                    out=s2c[:], in0=mc[:], scalar=1.0 / 127.0,
                    in1=comb_s[:, e, :], op0=mult, op1=mult),
            ]
            if pipelined:
                # w2sc only for the tail expert (its dequants are on the
                # critical path); fully on Pool, idle by now -- keeps DVE's
                # queue clear for the dequants. The other expert's dequant
                # applies w2s and s2c as two broadcast multiplies instead.
                ops += [
                    lambda: nc.gpsimd.tensor_tensor(
                        out=w2sc[:],
                        in0=w2s_s[:, e, :].unsqueeze(2).broadcast_to(
                            [128, HT, n_pad]),
                        in1=s2c[:].unsqueeze(1).broadcast_to([128, HT, n_pad]),
                        op=mult),
                ]
            if pipelined:
                return ops, []
            # interleaved expert: all ops ride the cast stream slots; each is
            # dep-ready ~1 slot after emission so they never head-of-line
            # block the casts
            return ops, []

        # --- GEMM1 pipeline: stream w1, cast int8->fp16, weight-stationary
        # --- matmuls; the previous expert's epilogue rides the cast stream
        g1_chunks = [(h, t) for h in (0, 1) for t in range(FT)]
        s_idx, e_idx = bank_flags([h * FW * 4 + t * n_pad * 4 for h, t in g1_chunks])
        g1_starts = {g1_chunks[i] for i in s_idx}
        g1_stops = {g1_chunks[i] for i in e_idx}

        epi_res = {e: {} for e in range(EPC)}
        pending, pending_tail = [], []
        for e in range(EPC):
            ps1_e = ps1_pool.tile([128, 2 * FW], F32, tag="ps1", name=f"ps1_{e}")
            for k in range(KH):
                w1i8 = w1i8p.tile([128, I2], I8, tag="w1i8", name=f"w1i8_{e}_{k}")
                nc.sync.dma_start(out=w1i8[:], in_=w1_d[e, k])
                w1f = w1f16p.tile([128, I2], F16, tag="w1f", name=f"w1f_{e}_{k}")
                o0, n0 = CAST_DVE
                nc.vector.tensor_copy(out=w1f[:, o0 : o0 + n0], in_=w1i8[:, o0 : o0 + n0])
                o1, n1 = CAST_ACT
                nc.scalar.activation(
                    out=w1f[:, o1 : o1 + n1],
                    in_=w1i8[:, o1 : o1 + n1],
                    func=mybir.ActivationFunctionType.Copy,
                )
                o2, n2 = CAST_POOL
                nc.gpsimd.tensor_copy(out=w1f[:, o2 : o2 + n2], in_=w1i8[:, o2 : o2 + n2])
                rhs = xqs_s[:, e, k, :]
                for half, t in g1_chunks:
                    base = half * FW
                    nc.tensor.matmul(
                        ps1_e[:, base + t * n_pad : base + (t + 1) * n_pad],
                        lhsT=w1f[:, half * I + t * 128 : half * I + (t + 1) * 128],
                        rhs=rhs,
                        start=(k == 0 and (half, t) in g1_starts),
                        stop=(k == KH - 1 and (half, t) in g1_stops),
                    )
                # sprinkle the previous expert's epilogue into this cast
                # stream so its cross-engine chain never stalls the casts
                if pending and k >= 1:
                    pending.pop(0)()
            # drain the previous expert's leftovers, then its scalar tail --
            # this lands right after this expert's cast stream in each
            # engine's queue, when the engines are freeing up
            for op in pending + pending_tail:
                op()
            pending, pending_tail = epilogue1_ops(
                e, ps1_e, epi_res[e], pipelined=(e == EPC - 1))
        # last expert's whole (pipelined) epilogue emitted inline
        for op in pending + pending_tail:
            op()

        # --- w2 DMA streams (SP queue, behind the full w1 stream) ---
        # All tiles stay resident (no pool rotation => DMAs never wait).
        # Expert 0: per-ki full tiles (728ns >= 625ns HWDGE => packed).
        # Expert 1: a 12-chunk-wide stream (2-ki DMAs, then ha) followed by a
        # 4-chunk tail stream (4+4+2 ki, then hb) so the final weight bytes
        # gate only HB matmuls + a small dequant + a 512B output DMA.
        w2f_tiles = []
        for ki in range(KI):
            t_ = w2fp.tile([128, H], F8E3, tag=f"w2f{ki}", name=f"w2f_{ki}")
            nc.sync.dma_start(out=t_[:], in_=w2_d[0, ki])
            w2f_tiles.append(t_)
        HAW = HA * 128
        a_chunks = [(k, min(2, KI - 1 - k)) for k in range(0, KI - 1, 2)]
        w2a_tiles = {}  # ki -> AP [128, HAW]
        for k0, nk in a_chunks:
            t_ = w2ap.tile([128, nk, HAW], F8E3, tag=f"w2a{k0}", name=f"w2a_{k0}")
            nc.sync.dma_start(
                out=t_[:],
                in_=w2_d[1, k0 : k0 + nk, :, 0:HAW].rearrange("k p h -> p k h"))
            for j in range(nk):
                w2a_tiles[k0 + j] = t_[:, j, :]
        ha = w2ap.tile([128, HAW], F8E3, tag="w2ha", name="w2ha")
        nc.sync.dma_start(out=ha[:], in_=w2_d[1, KI - 1][:, 0:HAW])
        w2a_tiles[KI - 1] = ha[:]
        b_chunks = [(0, 4), (4, 4), (8, 2)]
        w2b_tiles = {}  # ki -> AP [128, HB*128]
        for k0, nk in b_chunks:
            t_ = w2bp.tile([128, nk, HB * 128], F8E3, tag=f"w2b{k0}", name=f"w2b_{k0}")
            nc.sync.dma_start(
                out=t_[:],
                in_=w2_d[1, k0 : k0 + nk, :, HAW:H].rearrange("k p h -> p k h"))
            for j in range(nk):
                w2b_tiles[k0 + j] = t_[:, j, :]
        hb = w2bp.tile([128, HB * 128], F8E3, tag="w2hb", name="w2hb")
        nc.sync.dma_start(out=hb[:], in_=w2_d[1, KI - 1][:, HAW:H])
        w2b_tiles[KI - 1] = hb[:]

        # --- GEMM2 groups + dequants ---
        def gemm2_group(tiles, aq, psum, tcount, starts, stops):
            for ki in range(KI):
                for t in range(tcount):
                    nc.tensor.matmul(
                        psum[:, t * n_pad : (t + 1) * n_pad],
                        lhsT=tiles[ki][:, t * 128 : (t + 1) * 128],
                        rhs=aq[:, ki, :],
                        start=(ki == 0 and t in starts),
                        stop=(ki == KI - 1 and t in stops),
                    )

        full_starts, full_stops = bank_flags([t * n_pad * 4 for t in range(HT)])
        a_starts, a_stops = bank_flags([t * n_pad * 4 for t in range(HA)])
        b_starts, b_stops = bank_flags([t * n_pad * 4 for t in range(HB)])

        ps2f = ps2f_pool.tile([128, HT * n_pad], F32, tag="ps2f", name="ps2f")
        gemm2_group(w2f_tiles, epi_res[0]["aq"], ps2f, HT, full_starts, full_stops)
        # expert-0 dequant: two broadcast multiplies (w2s per-chunk, s2c
        # per-token) on the idle phase-2 DVE; avoids a w2sc precompute
        o_t = op_.tile([128, HT, n_pad], F32, tag="o_t", name="o_t")
        nc.vector.tensor_tensor(
            out=o_t[:],
            in0=ps2f[:].rearrange("p (t n) -> p t n", t=HT),
            in1=w2s_s[:, 0, :].unsqueeze(2).broadcast_to([128, HT, n_pad]),
            op=mult)
        o_f = op_.tile([128, HT * n_pad], BF16, tag="o_f", name="o_f")
        nc.vector.tensor_tensor(
            out=o_f[:].rearrange("p (t n) -> p t n", t=HT),
            in0=o_t[:],
            in1=epi_res[0]["s2c"][:].unsqueeze(1).broadcast_to([128, HT, n_pad]),
            op=mult)

        ps2a = ps2a_pool.tile([128, HA * n_pad], F32, tag="ps2a", name="ps2a")
        gemm2_group(w2a_tiles, epi_res[1]["aq"], ps2a, HA, a_starts, a_stops)
        o_a = op_.tile([128, HA * n_pad], BF16, tag="o_a", name="o_a")
        nc.vector.tensor_tensor(
            out=o_a[:], in0=ps2a[:],
            in1=epi_res[1]["w2sc"][:, 0:HA, :].rearrange("p t n -> p (t n)"),
            op=mult)

        ps2b = ps2b_pool.tile([128, HB * n_pad], F32, tag="ps2b", name="ps2b")
        gemm2_group(w2b_tiles, epi_res[1]["aq"], ps2b, HB, b_starts, b_stops)
        o_b = op_.tile([128, HB * n_pad], BF16, tag="o_b", name="o_b")
        nc.vector.tensor_tensor(
            out=o_b[:], in0=ps2b[:],
            in1=epi_res[1]["w2sc"][:, HA:HT, :].rearrange("p t n -> p (t n)"),
            op=mult)

        # --- output DMAs: queued on SP after all weight DMAs so their
        # --- transfers ride the tail's dead window
        nc.sync.dma_start(out=o_d[0], in_=o_f[:])
        nc.sync.dma_start(out=o_d[1, :, 0 : HA * n_pad], in_=o_a[:])
        nc.sync.dma_start(out=o_d[1, :, HA * n_pad : HT * n_pad], in_=o_b[:])

    nc.compile()
    return nc


def get_program(n_pad: int):
    key = ("nc", n_pad)
    if key not in _cache:
        _cache[key] = _build_program(n_pad)
    return _cache[key]


def _routing(expert_ids, expert_scales):
    """comb[B, E] scatter-add; token lists per expert; N_PAD."""
    comb = np.zeros((B, E), np.float32)
    np.add.at(comb, (np.arange(B)[:, None], np.asarray(expert_ids)),
              np.asarray(expert_scales, np.float32))
    routed = np.zeros((B, E), bool)
    routed[np.arange(B)[:, None], np.asarray(expert_ids)] = True
    toks = [np.nonzero(routed[:, e])[0] for e in range(E)]
    max_n = max(len(t) for t in toks)
    n_pad = 16
    while n_pad < max_n:
        n_pad *= 2
    # PSUM chunking requires pow2 n_pad; >64 would overflow the 8 banks
    assert n_pad <= 64, f"routing too dense for this kernel: n_pad={n_pad}"
    return comb, toks, n_pad


def _prep_inputs(x, expert_ids, smooth_scales, expert_scales, w1, w1_scale, w2, w2_scale):
    """Host-side dispatch: quantize x, route tokens, shard experts."""
    x = np.asarray(x, np.float32)
    smooth_scales = np.asarray(smooth_scales, np.float32)
    w1_scale = np.asarray(w1_scale, np.float32)
    w2_scale = np.asarray(w2_scale, np.float32)

    # dynamic per-token int8 quantization (exact mirror of reference ops)
    sx = np.maximum(np.max(np.abs(x), axis=-1, keepdims=True), 1e-12) / 127.0
    xq = np.round(np.clip(x / sx, -128.0, 127.0)).astype(np.float32)
    xqs = (xq * sx).astype(np.float16)  # [B, H]
    xqsT = np.ascontiguousarray(
        xqs.T.reshape(KH, 128, B).transpose(1, 0, 2)
    )  # [128, KH, B]

    comb, toks, n_pad = _routing(expert_ids, expert_scales)
    MC = 2 * FT + HT + n_pad

    w1v = np.asarray(w1).astype(np.int8)
    w2v = np.asarray(w2).astype(np.int8)

    in_maps = []
    for c in range(NCORES):
        es = list(range(c * EPC, (c + 1) * EPC))
        xqsT_e = np.zeros((EPC, 128, KH, n_pad), np.float16)
        comb_e = np.zeros((EPC, 128, n_pad), np.float32)
        for i, e in enumerate(es):
            tk = toks[e]
            xqsT_e[i, :, :, : len(tk)] = xqsT[:, :, tk]
            comb_e[i, :, : len(tk)] = comb[tk, e][None, :]
        w1c = w1v[es].reshape(EPC, KH, 128, I2)
        w2c = np.ascontiguousarray(
            (w2v[es].reshape(EPC, KI, 128, H).astype(np.float32) / 16.0)
        ).astype(ml_dtypes.float8_e3m4)
        # per-partition scale columns [e, p, T]
        s1g_full = w1_scale[es][:, :I]
        s1u_full = w1_scale[es][:, I:] * smooth_scales[es]
        s1g = np.ascontiguousarray(s1g_full.reshape(EPC, FT, 128).transpose(0, 2, 1))
        s1us = np.ascontiguousarray(
            s1u_full.reshape(EPC, FT, 128).transpose(0, 2, 1))
        sc2 = np.ascontiguousarray(
            (w2_scale[es] * 16.0).reshape(EPC, HT, 128).transpose(0, 2, 1))
        meta = np.zeros((EPC, 128, MC), np.float32)
        meta[:, :, MC_S1G : MC_S1G + FT] = s1g
        meta[:, :, MC_S1GU : MC_S1GU + FT] = s1us
        meta[:, :, MC_W2S : MC_W2S + HT] = sc2
        meta[:, :, MC_COMB : MC_COMB + n_pad] = comb_e
        in_maps.append(
            {
                "xqsT": xqsT_e,
                "w1t": np.ascontiguousarray(w1c),
                "w2t": w2c,
                "meta": meta,
            }
        )
    return in_maps, toks, n_pad


def kernel(
    x,
    expert_ids,
    smooth_scales,
    expert_scales,
    x_active_mask,
    w1,
    w1_scale,
    w2,
    w2_scale,
    _trace=False,
    _trace_kwargs=None,
):
    in_maps, toks, n_pad = _prep_inputs(
        x, expert_ids, smooth_scales, expert_scales, w1, w1_scale, w2, w2_scale
    )
    nc = get_program(n_pad)
    res = run_bass_kernel_spmd(
        nc,
        in_maps,
        core_ids=list(range(NCORES)),
        trace=_trace,
        **(_trace_kwargs or {}),
    )
    y = np.zeros((B, H), np.float32)
    for c, r in enumerate(res.results):
        o = np.asarray(r["o"], np.float32).reshape(EPC, 128, HT, n_pad)
        for i in range(EPC):
            e = c * EPC + i
            tk = toks[e]
            contrib = o[i, :, :, : len(tk)].transpose(2, 1, 0).reshape(len(tk), H)
            y[tk] += contrib
    y *= np.asarray(x_active_mask).astype(np.float32)[:, None]
    if _trace:
        kernel.last_results = res
    return y
